# revision 1
# baseline (speedup 1.0000x reference)
"""MsPoE Llama attention on 8 TRN2 NeuronCores (tensor-parallel over heads).

Strategy
--------
Heads are sharded 4-per-core. Two SPMD Bass launches:

  Launch A (per core): Q/K projections into head-transposed layout
    qT/kT [512, 2048], V projection in natural layout v [2048, 512]
    (fp32r on the PE at full rate), plus an *exact fp32* statistics path
    (q_last row, w = Wk_h^T q_last, srow = hs @ w) that yields the
    last-query attention-score row srow [4, 2048] used for the Ms-PoE
    head-outlier ordering. All stats matmuls are oriented so the
    stationary operand is tiny (<=4 columns) and the moving free dim is
    512 — they stream at full PE duty instead of thrashing LDWEIGHTS.

  Host glue: softmax/count/argsort of the 32 head outlier counts (f64;
    margins are ~3e-5, so fp32-exact device scores decide identically
    to the reference), then builds the head-order-permuted per-head
    RoPE cos/sin caches.

  Launch B (per core): per-head RoPE (DVE), causal attention per head
    in transposed layout (scores^T = k^T q per 128-key tile, exp on
    ACT, denominator via ones-matmul on the PE), unnormalized
    attention rows are copied out of PSUM immediately (frees banks for
    the next block), normalization happens once at the end via one
    batched reciprocal + PE broadcast, then the o_proj partial
    oT [4096, 2048].

  Host: sum the 8 o_proj partials -> [1, 2048, 4096].

All heavy FLOPs run on the NeuronCores; the host only does the 32-way
argsort (control flow), constant cos/sin table construction, and the
final 8-way partial-sum (unsharding).
"""

import os
import sys

import numpy as np

for _p in ("/opt/trn_rl_repo", "/root/.axon_site/_ro/trn_rl_repo"):
    if os.path.isdir(_p) and _p not in sys.path:
        sys.path.append(_p)

import concourse.bass as bass  # noqa: E402
import concourse.tile as tile  # noqa: E402
from concourse import bacc, mybir  # noqa: E402
from concourse import bass_utils  # noqa: E402

F32 = mybir.dt.float32
MM_DT = mybir.dt.float32r  # main-path matmul dtype (fp32r: full PE rate, ~1.5e-4)

B, S, HID, H, D = 1, 2048, 4096, 32, 128
NCORES, HPC = 8, 4          # cores, heads per core
JC = HPC * D                # 512: per-core projection width
KT = HID // 128             # 32 contraction tiles
SB = S // 512               # 4 sequence blocks
EB = 2                      # e-tiles per DMA batch in the projection streams
BASE, MIN_R, MAX_R = 10000.0, 1.0, 3.0
SCALE = 1.0 / float(np.sqrt(D))
NEGM = -1.0e35              # additive causal mask value (exp -> 0)

_CACHE = {}
TRACE = False          # set True (e.g. from test.py) to profile the launches
LAST_PROFILE = {}      # filled with BassKernelResults when TRACE is on


def build_A():
    nc = bacc.Bacc("TRN2", target_bir_lowering=False, debug=False, num_devices=NCORES)
    hsT = nc.dram_tensor("hsT", [HID, S], F32, kind="ExternalInput").ap()
    wqT = nc.dram_tensor("wqT", [HID, JC], F32, kind="ExternalInput").ap()
    wkT = nc.dram_tensor("wkT", [HID, JC], F32, kind="ExternalInput").ap()
    wvT = nc.dram_tensor("wvT", [HID, JC], F32, kind="ExternalInput").ap()
    wkN = nc.dram_tensor("wkN", [JC, HID], F32, kind="ExternalInput").ap()
    qT = nc.dram_tensor("qT", [JC, S], F32, kind="ExternalOutput").ap()
    kT = nc.dram_tensor("kT", [JC, S], F32, kind="ExternalOutput").ap()
    vN = nc.dram_tensor("vN", [S, JC], F32, kind="ExternalOutput").ap()
    srow = nc.dram_tensor("srow", [HPC, S], F32, kind="ExternalOutput").ap()

    # batched views: e-tiles grouped EB at a time
    hsT_b = hsT.rearrange("(eb g p) s -> p eb g s", p=128, g=EB)   # [128, KT/EB, EB, S]
    wqT_b = wqT.rearrange("(eb g p) j -> p eb g j", p=128, g=EB)
    wkT_b = wkT.rearrange("(eb g p) j -> p eb g j", p=128, g=EB)
    wvT_b = wvT.rearrange("(eb g p) j -> p eb g j", p=128, g=EB)
    qT_b = qT.rearrange("(jt p) s -> p jt s", p=128)               # [128, 4, S]
    kT_b = kT.rearrange("(jt p) s -> p jt s", p=128)
    vN_b = vN.rearrange("(t p) j -> p t j", p=128)                 # [128, 16, JC]
    NB = KT // EB

    with tile.TileContext(nc) as tc:
        with (
            tc.tile_pool(name="wpool", bufs=2) as wpool,     # persistent W (f32r)
            tc.tile_pool(name="wtmp", bufs=2) as wtmp,       # f32 staging for W
            tc.tile_pool(name="hpool", bufs=3) as hpool,     # hsT stream tiles (f32)
            tc.tile_pool(name="hconv", bufs=3) as hconv,     # converted stream tiles
            tc.tile_pool(name="small", bufs=1) as small,
            tc.tile_pool(name="outp", bufs=2) as outp,
            tc.tile_pool(name="dramp", bufs=1, space="DRAM") as dramp,
            tc.tile_pool(name="ps", bufs=8, space="PSUM") as ps,
        ):
            # hs last column (for exact q_last): [128, KT]
            hsl = small.tile([128, KT], F32)
            nc.sync.dma_start(
                hsl, hsT[:, S - 1 : S].rearrange("(kt p) one -> p (kt one)", p=128)
            )

            # ---- Wq: load + convert; piggyback exact fp32 q_last row ----
            # q_last^T chunk: lhsT = hs_last column (1-wide stationary),
            # rhs = Wq^T stage tile -> [1, 512] psum accumulated over e.
            wq_m = wpool.tile([128, KT, JC], MM_DT, tag="w")
            ps_ql = ps.tile([1, JC], F32, tag="ps")
            for eb in range(NB):
                t = wtmp.tile([128, EB, JC], F32, tag="wstage")
                nc.sync.dma_start(t, wqT_b[:, eb])
                nc.vector.tensor_copy(wq_m[:, eb * EB : (eb + 1) * EB], t)
                for g in range(EB):
                    e = eb * EB + g
                    nc.tensor.matmul(
                        ps_ql, hsl[:, e : e + 1], t[:, g],
                        start=(e == 0), stop=(e == KT - 1),
                    )
            qlrow = outp.tile([1, JC], F32, tag="row")
            nc.scalar.copy(qlrow, ps_ql)
            # roundtrip through DRAM to get q_last as per-head columns [128, 4]
            ql_d = dramp.tile([1, JC], F32)
            nc.sync.dma_start(ql_d, qlrow)
            qlcol = small.tile([128, HPC], F32)
            nc.sync.dma_start(
                qlcol, ql_d.rearrange("one (h d) -> (one d) h", d=128)
            )

            # ---- Wk: load + convert ----
            wk_m = wpool.tile([128, KT, JC], MM_DT, tag="w")
            for eb in range(NB):
                t = wtmp.tile([128, EB, JC], F32, tag="wstage")
                nc.sync.dma_start(t, wkT_b[:, eb])
                nc.vector.tensor_copy(wk_m[:, eb * EB : (eb + 1) * EB], t)

            # ---- pass 1: Q/K projections (transposed layout) ----
            for sb in range(SB - 1):
                ss = slice(sb * 512, (sb + 1) * 512)
                ps_q = [ps.tile([128, 512], F32, tag="ps", name=f"ps_q{sb}_{i}")
                        for i in range(HPC)]
                ps_k = [ps.tile([128, 512], F32, tag="ps", name=f"ps_k{sb}_{i}")
                        for i in range(HPC)]
                for eb in range(NB):
                    hst = hpool.tile([128, EB, 512], F32, tag="h")
                    nc.sync.dma_start(hst, hsT_b[:, eb, :, ss])
                    hm = hconv.tile([128, EB, 512], MM_DT, tag="conv")
                    nc.vector.tensor_copy(hm, hst)
                    for g in range(EB):
                        e = eb * EB + g
                        for jt in range(HPC):
                            js = slice(jt * 128, (jt + 1) * 128)
                            nc.tensor.matmul(
                                ps_q[jt], wq_m[:, e, js], hm[:, g],
                                start=(e == 0), stop=(e == KT - 1),
                            )
                            nc.tensor.matmul(
                                ps_k[jt], wk_m[:, e, js], hm[:, g],
                                start=(e == 0), stop=(e == KT - 1),
                            )
                oq = outp.tile([128, HPC, 512], F32, tag="obig")
                ok_ = outp.tile([128, HPC, 512], F32, tag="obig")
                for jt in range(HPC):
                    nc.scalar.copy(oq[:, jt], ps_q[jt])
                    nc.scalar.copy(ok_[:, jt], ps_k[jt])
                nc.sync.dma_start(qT_b[:, :, ss], oq)
                nc.sync.dma_start(kT_b[:, :, ss], ok_)

            # ---- stats w rows: w_h = Wk_h^T q_last_h (fp32, 512-wide streams)
            wrow_d = dramp.tile([HPC, HID], F32)
            for h in range(HPC):
                for qt in range(4):
                    kn = wtmp.tile([128, HID // 4], F32, tag="wkn")
                    nc.sync.dma_start(
                        kn,
                        wkN[h * 128 : (h + 1) * 128,
                            qt * (HID // 4) : (qt + 1) * (HID // 4)],
                    )
                    for c in range(HID // 4 // 512):
                        ps_w = ps.tile([1, 512], F32, tag="ps", name=f"psw{h}_{qt}_{c}")
                        nc.tensor.matmul(
                            ps_w, qlcol[:, h : h + 1],
                            kn[:, c * 512 : (c + 1) * 512],
                            start=True, stop=True,
                        )
                        wr = outp.tile([1, 512], F32, tag="row")
                        nc.scalar.copy(wr, ps_w)
                        nc.sync.dma_start(
                            wrow_d[h, (qt * 2 + c) * 512 : (qt * 2 + c + 1) * 512],
                            wr,
                        )
            # read w back as per-e-tile columns [128, KT, HPC]
            wcol = small.tile([128, KT, HPC], F32)
            for h in range(HPC):
                nc.sync.dma_start(
                    wcol[:, :, h : h + 1],
                    wrow_d[h : h + 1, :].rearrange(
                        "one (kt p) -> p kt one", p=128
                    ),
                )

            # ---- pass 1: Q/K projections (transposed layout) ----
            for sb in [SB - 1]:
                ss = slice(sb * 512, (sb + 1) * 512)
                ps_q = [ps.tile([128, 512], F32, tag="ps", name=f"ps_qz{sb}_{i}")
                        for i in range(HPC)]
                ps_k = [ps.tile([128, 512], F32, tag="ps", name=f"ps_kz{sb}_{i}")
                        for i in range(HPC)]
                for eb in range(NB):
                    hst = hpool.tile([128, EB, 512], F32, tag="h")
                    nc.sync.dma_start(hst, hsT_b[:, eb, :, ss])
                    hm = hconv.tile([128, EB, 512], MM_DT, tag="conv")
                    nc.vector.tensor_copy(hm, hst)
                    for g in range(EB):
                        e = eb * EB + g
                        for jt in range(HPC):
                            js = slice(jt * 128, (jt + 1) * 128)
                            nc.tensor.matmul(
                                ps_q[jt], wq_m[:, e, js], hm[:, g],
                                start=(e == 0), stop=(e == KT - 1),
                            )
                            nc.tensor.matmul(
                                ps_k[jt], wk_m[:, e, js], hm[:, g],
                                start=(e == 0), stop=(e == KT - 1),
                            )
                oq = outp.tile([128, HPC, 512], F32, tag="obig")
                ok_ = outp.tile([128, HPC, 512], F32, tag="obig")
                for jt in range(HPC):
                    nc.scalar.copy(oq[:, jt], ps_q[jt])
                    nc.scalar.copy(ok_[:, jt], ps_k[jt])
                nc.sync.dma_start(qT_b[:, :, ss], oq)
                nc.sync.dma_start(kT_b[:, :, ss], ok_)

            # ---- Wv: load + convert (reuses a wpool slot once Wq frees) ----
            wv_m = wpool.tile([128, KT, JC], MM_DT, tag="w")
            for eb in range(NB):
                t = wtmp.tile([128, EB, JC], F32, tag="wstage")
                nc.sync.dma_start(t, wvT_b[:, eb])
                nc.vector.tensor_copy(wv_m[:, eb * EB : (eb + 1) * EB], t)

            # ---- pass 2: exact srow (emitted first: keeps PE fed while Wv
            # loads) + V projection (natural layout) ----
            for sb in range(SB):
                ss = slice(sb * 512, (sb + 1) * 512)
                ps_sr = ps.tile([4, 512], F32, tag="ps", name=f"ps_sr{sb}")
                ps_v = [ps.tile([128, 512], F32, tag="ps", name=f"ps_v{sb}_{i}")
                        for i in range(4)]
                for eb in range(NB):
                    hst = hpool.tile([128, EB, 512], F32, tag="h")
                    nc.sync.dma_start(hst, hsT_b[:, eb, :, ss])
                    hm = hconv.tile([128, EB, 512], MM_DT, tag="conv")
                    nc.vector.tensor_copy(hm, hst)
                    for g in range(EB):
                        e = eb * EB + g
                        nc.tensor.matmul(
                            ps_sr, wcol[:, e], hst[:, g],
                            start=(e == 0), stop=(e == KT - 1),
                        )
                        for t4 in range(4):
                            cs = slice(t4 * 128, (t4 + 1) * 128)
                            nc.tensor.matmul(
                                ps_v[t4], hm[:, g, cs], wv_m[:, e],
                                start=(e == 0), stop=(e == KT - 1),
                            )
                osr = outp.tile([4, 512], F32, tag="row")
                nc.scalar.copy(osr, ps_sr)
                nc.sync.dma_start(srow[:, ss], osr)
                ov = outp.tile([128, 4, 512], F32, tag="obig")
                for t4 in range(4):
                    nc.scalar.copy(ov[:, t4], ps_v[t4])
                nc.sync.dma_start(vN_b[:, sb * 4 : (sb + 1) * 4, :], ov)

    nc.compile()
    return nc


def build_B():
    nc = bacc.Bacc("TRN2", target_bir_lowering=False, debug=False, num_devices=NCORES)
    qT = nc.dram_tensor("qT", [JC, S], F32, kind="ExternalInput").ap()
    kT = nc.dram_tensor("kT", [JC, S], F32, kind="ExternalInput").ap()
    vN = nc.dram_tensor("vN", [S, JC], F32, kind="ExternalInput").ap()
    cosT = nc.dram_tensor("cosT", [JC, S], F32, kind="ExternalInput").ap()
    shatT = nc.dram_tensor("shatT", [JC, S], F32, kind="ExternalInput").ap()
    woT = nc.dram_tensor("woT", [JC, HID], F32, kind="ExternalInput").ap()
    masks = nc.dram_tensor("masks", [128, 4 * 512], F32, kind="ExternalInput").ap()
    oT = nc.dram_tensor("oT", [HID, S], F32, kind="ExternalOutput").ap()

    vN_b = vN.rearrange("(t p) j -> p t j", p=128)   # [128, 16, JC]
    woT_b = woT.rearrange("(jt p) e -> p jt e", p=128)
    oT_b = oT.rearrange("(et p) s -> p et s", p=128)

    with tile.TileContext(nc) as tc:
        with (
            tc.tile_pool(name="vres", bufs=2) as v_pool,     # per-head V (f32r)
            tc.tile_pool(name="attn", bufs=1) as attn_pool,  # unnorm attn (f32r)
            tc.tile_pool(name="stage", bufs=2) as stage,
            tc.tile_pool(name="ld", bufs=12) as ld,          # q/k/cos/sin chunks f32
            tc.tile_pool(name="rope", bufs=4) as rope,       # rq/rk f32r
            tc.tile_pool(name="rtmp", bufs=4) as rtmp,
            tc.tile_pool(name="expp", bufs=4) as expp,       # exp tiles f32r
            tc.tile_pool(name="small", bufs=1) as small,
            tc.tile_pool(name="zp", bufs=2) as zp,
            tc.tile_pool(name="wo", bufs=3) as wo_pool,      # streamed Wo tiles
            tc.tile_pool(name="outp", bufs=3) as outp,
            tc.tile_pool(name="dramp", bufs=1, space="DRAM") as dramp,
            tc.tile_pool(name="ps", bufs=8, space="PSUM") as ps,
        ):
            # constants
            masks_sb = small.tile([128, 4, 512], F32)
            nc.sync.dma_start(masks_sb, masks.rearrange("p (r j) -> p r j", r=4))
            onesf_c = small.tile([128, 1], F32)
            nc.vector.memset(onesf_c, 1.0)
            ones_col = small.tile([128, 1], MM_DT)
            nc.vector.tensor_copy(ones_col, onesf_c)

            attnT = attn_pool.tile([128, HPC * SB, 512], MM_DT)
            zAll = small.tile([HPC * SB, 512], F32)

            ropes = {}
            vs = {}

            def v_load(h):
                rs_ = slice(h * 128, (h + 1) * 128)
                vstg = stage.tile([128, S // 128, 128], F32, tag="vstage",
                                  name=f"vstg{h}")
                nc.sync.dma_start(vstg, vN_b[:, :, rs_])
                vm = v_pool.tile([128, S // 128, 128], MM_DT, tag="v",
                                 name=f"vm{h}")
                nc.vector.tensor_copy(vm, vstg)
                vs[h] = vm

            def rope_chunk(h, cb):
                rs_ = slice(h * 128, (h + 1) * 128)
                lo = slice(h * 128, h * 128 + 64)
                hi = slice(h * 128 + 64, h * 128 + 128)
                if h not in ropes:
                    ropes[h] = (
                        rope.tile([128, S], MM_DT, tag="rope", name=f"rq{h}"),
                        rope.tile([128, S], MM_DT, tag="rope", name=f"rk{h}"),
                    )
                rq_, rk_ = ropes[h]
                cs = slice(cb * 512, (cb + 1) * 512)
                ch = ld.tile([128, 512], F32, tag="ld", name=f"ch{h}_{cb}")
                nc.sync.dma_start(ch, cosT[rs_, cs])
                sh = ld.tile([128, 512], F32, tag="ld", name=f"sh{h}_{cb}")
                nc.sync.dma_start(sh, shatT[rs_, cs])
                qh = ld.tile([128, 512], F32, tag="ld", name=f"qh{h}_{cb}")
                nc.sync.dma_start(qh, qT[rs_, cs])
                kh = ld.tile([128, 512], F32, tag="ld", name=f"kh{h}_{cb}")
                nc.sync.dma_start(kh, kT[rs_, cs])
                qsw = ld.tile([128, 512], F32, tag="ld", name=f"qsw{h}_{cb}")
                nc.sync.dma_start(qsw[0:64, :], qT[hi, cs])
                nc.sync.dma_start(qsw[64:128, :], qT[lo, cs])
                ksw = ld.tile([128, 512], F32, tag="ld", name=f"ksw{h}_{cb}")
                nc.sync.dma_start(ksw[0:64, :], kT[hi, cs])
                nc.sync.dma_start(ksw[64:128, :], kT[lo, cs])
                for dst, src_, ssw in ((rq_, qh, qsw), (rk_, kh, ksw)):
                    d = dst[:, cs]
                    nc.vector.tensor_mul(d, src_, ch)
                    t2 = rtmp.tile([128, 512], F32, tag="rt")
                    nc.vector.tensor_mul(t2, ssw, sh)
                    nc.vector.tensor_add(d, d, t2)

            v_load(0)
            for cb in range(SB):
                rope_chunk(0, cb)
            v_load(1)

            for h in range(HPC):
                v_m = vs[h]
                rq, rk = ropes[h]
                for qb in range(SB):
                    nkt = 4 * qb + 4
                    qs = slice(qb * 512, (qb + 1) * 512)
                    i16 = h * SB + qb
                    ps_o = ps.tile([128, 512], F32, tag="ps")
                    ps_z = ps.tile([1, 512], F32, tag="ps")
                    for kt in range(nkt):
                        ps_s = ps.tile([128, 512], F32, tag="ps")
                        nc.tensor.matmul(
                            ps_s, rk[:, kt * 128 : (kt + 1) * 128], rq[:, qs],
                            start=True, stop=True,
                        )
                        r = kt - 4 * qb
                        if r >= 0:
                            nc.vector.tensor_add(ps_s, ps_s, masks_sb[:, r])
                        ext = expp.tile([128, 512], MM_DT, tag="exp")
                        nc.scalar.activation(
                            ext, ps_s, mybir.ActivationFunctionType.Exp, scale=SCALE
                        )
                        nc.tensor.matmul(
                            ps_z, ones_col, ext,
                            start=(kt == 0), stop=(kt == nkt - 1),
                        )
                        nc.tensor.matmul(
                            ps_o, v_m[:, kt], ext,
                            start=(kt == 0), stop=(kt == nkt - 1),
                        )
                    # drain PSUM fast: stash unnormalized rows + Z, free banks
                    ztmp = zp.tile([1, 512], F32, tag="zr")
                    nc.scalar.copy(ztmp, ps_z)
                    nc.sync.dma_start(zAll[i16 : i16 + 1, :], ztmp)
                    nc.vector.tensor_copy(attnT[:, i16], ps_o)
                    if h + 1 < HPC:
                        rope_chunk(h + 1, qb)
                if h + 2 < HPC:
                    v_load(h + 2)

            # ---- batched softmax denominators + in-place normalization ----
            # 1/Z in full fp32, partition-broadcast via a DRAM roundtrip.
            zAllr = small.tile([HPC * SB, 512], F32)
            nc.vector.reciprocal(zAllr, zAll)
            zr_d = dramp.tile([HPC * SB, 512], F32)
            nc.sync.dma_start(zr_d, zAllr)
            for sb in range(SB):
                for jt in range(HPC):
                    i16 = jt * SB + sb
                    zb = zp.tile([128, 512], F32, tag="zb")
                    nc.sync.dma_start(
                        zb, zr_d[i16 : i16 + 1, :].to_broadcast((128, 512))
                    )
                    with nc.allow_low_precision(reason="attn normalize writes f32r"):
                        nc.vector.tensor_tensor(
                            attnT[:, i16], attnT[:, i16], zb,
                            op=mybir.AluOpType.mult,
                        )

            # ---- O-projection partials: oT[e, s] = sum_h Wo_h^T attn_h ----
            # Wo is streamed: each [128, HPC, 128] column-tile serves all
            # four s-blocks, then is discarded.
            for et in range(KT):
                wstg = wo_pool.tile([128, HPC, 128], F32, tag="wostage")
                nc.sync.dma_start(wstg, woT_b[:, :, et * 128 : (et + 1) * 128])
                wo_et = wo_pool.tile([128, HPC, 128], MM_DT, tag="wo")
                nc.vector.tensor_copy(wo_et, wstg)
                oo = outp.tile([128, SB, 512], F32, tag="oo")
                for sb in range(SB):
                    ps_oo = ps.tile([128, 512], F32, tag="ps")
                    for jt in range(HPC):
                        nc.tensor.matmul(
                            ps_oo,
                            wo_et[:, jt],
                            attnT[:, jt * SB + sb],
                            start=(jt == 0),
                            stop=(jt == HPC - 1),
                        )
                    if (et * SB + sb) % 2 == 0:
                        nc.scalar.copy(oo[:, sb], ps_oo)
                    else:
                        nc.vector.tensor_copy(oo[:, sb], ps_oo)
                nc.sync.dma_start(oT_b[:, et, :], oo)

    nc.compile()
    return nc


def _get_nc(which):
    if which not in _CACHE:
        _CACHE[which] = build_A() if which == "A" else build_B()
    return _CACHE[which]


def _causal_mask_templates():
    # masked (NEGM) iff 128*r + p > j for p in [0,128), j in [0,512)
    p = np.arange(128)[:, None]
    j = np.arange(512)[None, :]
    out = np.zeros((128, 4, 512), np.float32)
    for r in range(4):
        out[:, r, :] = np.where(128 * r + p > j, NEGM, 0.0).astype(np.float32)
    return np.ascontiguousarray(out.reshape(128, 4 * 512))


def _rope_cache_np():
    # mirrors reference._rope_cache in float32
    inv_freq = (1.0 / (BASE ** (np.arange(0, D, 2, dtype=np.float32) / np.float32(D)))).astype(np.float32)
    ratio = (MIN_R + (MAX_R - MIN_R) * (np.arange(H, dtype=np.float32) / np.float32(H))).astype(np.float32)
    t = (np.arange(S, dtype=np.float32)[None, :] / ratio[:, None]).astype(np.float32)
    freqs = (t[:, :, None] * inv_freq[None, None, :]).astype(np.float32)
    emb = np.concatenate([freqs, freqs], axis=-1)
    return np.cos(emb).astype(np.float32), np.sin(emb).astype(np.float32)


def kernel(hidden_states, position_ids, Wq, Wk, Wv, Wo):
    hs = np.asarray(hidden_states, dtype=np.float32)[0]        # [S, HID]
    pos = np.asarray(position_ids).astype(np.int64)[0]         # [S]
    Wq = np.asarray(Wq, dtype=np.float32)
    Wk = np.asarray(Wk, dtype=np.float32)
    Wv = np.asarray(Wv, dtype=np.float32)
    Wo = np.asarray(Wo, dtype=np.float32)

    hsT = np.ascontiguousarray(hs.T)                           # [HID, S]

    ncA = _get_nc("A")
    in_maps_A = []
    for c in range(NCORES):
        rows = slice(c * JC, (c + 1) * JC)
        in_maps_A.append(
            {
                "hsT": hsT,
                "wqT": np.ascontiguousarray(Wq[rows, :].T),
                "wkT": np.ascontiguousarray(Wk[rows, :].T),
                "wvT": np.ascontiguousarray(Wv[rows, :].T),
                "wkN": np.ascontiguousarray(Wk[rows, :]),
            }
        )
    resA = bass_utils.run_bass_kernel_spmd(
        ncA, in_maps_A, core_ids=list(range(NCORES)), trace=TRACE
    )
    if TRACE:
        LAST_PROFILE["A"] = resA

    # ---- host: head outlier counts -> head order (exact control flow) ----
    srow = np.concatenate(
        [resA.results[c]["srow"] for c in range(NCORES)], axis=0
    )  # [H, S]
    sc = srow.astype(np.float64) * SCALE
    m = sc.max(axis=-1, keepdims=True)
    e = np.exp(sc - m)
    aw = e / e.sum(axis=-1, keepdims=True)
    avg = aw.mean(axis=-1, keepdims=True)
    cnt = (aw > 3.0 * avg).sum(axis=-1)
    outlier = (-(cnt / np.float32(S))).astype(np.float32)
    head_order = np.argsort(outlier, kind="stable")

    # ---- host: permuted per-head RoPE caches (constants + gather) ----
    cos, sin = _rope_cache_np()
    cos_o = cos[head_order][:, pos, :]                         # [H, S, D]
    sin_o = sin[head_order][:, pos, :]
    masks = _causal_mask_templates()

    ncB = _get_nc("B")
    in_maps_B = []
    for c in range(NCORES):
        ct = np.ascontiguousarray(
            np.concatenate([cos_o[c * HPC + i].T for i in range(HPC)], axis=0)
        )  # [JC, S]
        st = np.concatenate([sin_o[c * HPC + i].T for i in range(HPC)], axis=0)
        st = st.copy()
        for i in range(HPC):
            st[i * D : i * D + D // 2, :] *= -1.0
        cols = slice(c * JC, (c + 1) * JC)
        in_maps_B.append(
            {
                "qT": resA.results[c]["qT"],
                "kT": resA.results[c]["kT"],
                "vN": resA.results[c]["vN"],
                "cosT": ct,
                "shatT": np.ascontiguousarray(st),
                "woT": np.ascontiguousarray(Wo[:, cols].T),
                "masks": masks,
            }
        )
    resB = bass_utils.run_bass_kernel_spmd(
        ncB, in_maps_B, core_ids=list(range(NCORES)), trace=TRACE
    )
    if TRACE:
        LAST_PROFILE["B"] = resB

    # ---- host: unshard (sum o_proj partials) ----
    acc = np.zeros((HID, S), np.float64)
    for c in range(NCORES):
        acc += resB.results[c]["oT"]
    return np.ascontiguousarray(acc.T)[None, :, :].astype(np.float32)



# revision 8
# speedup vs baseline: 1.2939x; 1.2939x over previous
"""MsPoE Llama attention on 8 TRN2 NeuronCores (tensor-parallel over heads).

Strategy (v2: single launch)
----------------------------
The reference's head-ordering statistic only needs the LAST pre-RoPE
attention row: srow_h = q_last_h . k_h[s]. By associativity,
srow_h = hs @ (Wk_h^T (Wq_h hs[-1])) — ~0.5 GFLOP, computed on the
host in float64 BEFORE launching (verified to reproduce the reference
head_order exactly: min margin to the 3*avg threshold is ~1e-5 vs
~6e-7 fp32-path noise). With head_order known up-front, the permuted
per-head RoPE cos/sin caches become plain inputs and the whole module
runs in ONE device launch with q/k/v resident in SBUF:

  Per core (4 heads), all matmul operands bf16 (1 PE cycle/row — the
  same rate as fp32r — but half the DMA/SBUF, and no f32->f32r
  conversion casts at all):

  1. QK pass: stream hsT once, accumulate q/k head-blocks in PSUM
     (8 banks), apply RoPE directly out of PSUM on DVE (+gpsimd for
     one swap-half) into resident rq/rk [128, 4, 2048] bf16 tiles.
     rotate_half's partition swap is two half-partition reads at
     offset 64/0; the sin sign flip is pre-applied on the host (shat).
  2. V pass: stream hsT again (wq+wk+wv cannot be resident at once),
     natural-layout V into resident v_m [128, 16, 512] bf16.
  3. Attention per (qb outer, head inner): scores^T = rk_chunk^T rq
     per 128-key tile, exp on ACT (bf16 out), softmax denominator via
     ones-matmul accumulation, unnormalized AV accumulation; 1/z is
     partition-broadcast with a tiny PE outer-product (no DRAM
     roundtrip); attnT normalized in place (DVE).
  4. o_proj for qb is emitted right after qb's heads finish, so its
     matmuls overlap the next qb's attention chain; partials oT
     [4096, 2048] f32 stream out per 512-column block.

  Host: sum the 8 o_proj partials (f64) -> [1, 2048, 4096].
"""

import os
import sys

import numpy as np

for _p in ("/opt/trn_rl_repo", "/root/.axon_site/_ro/trn_rl_repo"):
    if os.path.isdir(_p) and _p not in sys.path:
        sys.path.append(_p)

import concourse.bass as bass  # noqa: E402
import concourse.tile as tile  # noqa: E402
from concourse import bacc, mybir  # noqa: E402
from concourse import bass_utils  # noqa: E402

import ml_dtypes  # noqa: E402

F32 = mybir.dt.float32
BF16 = mybir.dt.bfloat16
F32R = mybir.dt.float32r
NPBF16 = ml_dtypes.bfloat16

B, S, HID, H, D = 1, 2048, 4096, 32, 128
NCORES, HPC = 8, 4          # cores, heads per core
JC = HPC * D                # 512: per-core projection width
KT = HID // 128             # 32 contraction tiles
SB = S // 512               # 4 sequence blocks
EB = 4                      # e-tiles per hs-stream DMA
BASE, MIN_R, MAX_R = 10000.0, 1.0, 3.0
SCALE = 1.0 / float(np.sqrt(D))
NEGM = -1.0e35              # additive causal mask value (exp -> 0)

_CACHE = {}
TRACE = False          # set True (e.g. from test.py) to profile the launch
LAST_PROFILE = {}      # filled with BassKernelResults when TRACE is on


def build():
    nc = bacc.Bacc("TRN2", target_bir_lowering=False, debug=False, num_devices=NCORES)
    hsT = nc.dram_tensor("hsT", [HID, S], BF16, kind="ExternalInput").ap()
    wqT = nc.dram_tensor("wqT", [HID, JC], BF16, kind="ExternalInput").ap()
    wkT = nc.dram_tensor("wkT", [HID, JC], BF16, kind="ExternalInput").ap()
    wvT = nc.dram_tensor("wvT", [HID, JC], BF16, kind="ExternalInput").ap()
    woT = nc.dram_tensor("woT", [JC, HID], BF16, kind="ExternalInput").ap()
    cosT = nc.dram_tensor("cosT", [JC, S], BF16, kind="ExternalInput").ap()
    shatT = nc.dram_tensor("shatT", [JC, S], BF16, kind="ExternalInput").ap()
    masks = nc.dram_tensor("masks", [128, 4 * 512], F32, kind="ExternalInput").ap()
    oT = nc.dram_tensor("oT", [HID, S], F32, kind="ExternalOutput").ap()

    hsT_b = hsT.rearrange("(eb g p) s -> p eb g s", p=128, g=EB)   # [128, 8, EB, S]
    wqT_b = wqT.rearrange("(kt p) j -> p kt j", p=128)             # [128, 32, JC]
    wkT_b = wkT.rearrange("(kt p) j -> p kt j", p=128)
    wvT_b = wvT.rearrange("(kt p) j -> p kt j", p=128)
    woT_b = woT.rearrange("(jt p) e -> p jt e", p=128)             # [128, 4, HID]
    cosT_b = cosT.rearrange("(h p) s -> p h s", p=128)             # [128, 4, S]
    shatT_b = shatT.rearrange("(h p) s -> p h s", p=128)
    oT_b = oT.rearrange("(et p) s -> p et s", p=128)               # [128, 32, S]
    NB = KT // EB

    with tile.TileContext(nc) as tc:
        with (
            tc.tile_pool(name="wres", bufs=2) as wres,       # wq/wk then wv/wo
            tc.tile_pool(name="trig", bufs=2) as trig,       # cos/shat resident
            tc.tile_pool(name="big", bufs=4) as big,         # rq/rk/v/attnT resident
            tc.tile_pool(name="hpool", bufs=2) as hpool,     # hs stream tiles
            tc.tile_pool(name="rtmp", bufs=5) as rtmp,       # RoPE f32 temps
            tc.tile_pool(name="expp", bufs=3) as expp,       # exp tiles bf16
            tc.tile_pool(name="small", bufs=1) as small,
            tc.tile_pool(name="zp", bufs=4) as zp,           # 1/z rows f32r
            tc.tile_pool(name="outp", bufs=3) as outp,
            tc.tile_pool(name="ps", bufs=8, space="PSUM") as ps,
        ):
            # ---- constants ----
            masks_sb = small.tile([128, 4, 512], F32)
            nc.sync.dma_start(masks_sb, masks.rearrange("p (r j) -> p r j", r=4))
            onesf = small.tile([128, 1], F32)
            nc.vector.memset(onesf, 1.0)
            ones_bf = small.tile([128, 1], BF16)
            nc.vector.tensor_copy(ones_bf, onesf)
            onesf_r = small.tile([1, 128], F32)
            nc.vector.memset(onesf_r, 1.0)
            ones_row = small.tile([1, 128], F32R)
            nc.vector.tensor_copy(ones_row, onesf_r)

            # ---- resident loads ----
            wq_m = wres.tile([128, KT, JC], BF16, tag="w", name="wq")
            wk_m = wres.tile([128, KT, JC], BF16, tag="w", name="wk")
            for c4 in range(4):
                cs = slice(c4 * 8, (c4 + 1) * 8)
                nc.sync.dma_start(wq_m[:, cs], wqT_b[:, cs])
                nc.sync.dma_start(wk_m[:, cs], wkT_b[:, cs])
            cos_m = trig.tile([128, HPC, S], BF16, tag="t", name="cos")
            shat_m = trig.tile([128, HPC, S], BF16, tag="t", name="shat")
            nc.sync.dma_start(cos_m, cosT_b)
            nc.sync.dma_start(shat_m, shatT_b)

            rq = big.tile([128, HPC, S], BF16, tag="big", name="rq")
            rk = big.tile([128, HPC, S], BF16, tag="big", name="rk")

            # ---- phase 1: QK projections + fused RoPE ----
            for sb in range(SB):
                ss = slice(sb * 512, (sb + 1) * 512)
                ps_q = [ps.tile([128, 512], F32, tag="ps", name=f"psq{sb}_{i}")
                        for i in range(HPC)]
                ps_k = [ps.tile([128, 512], F32, tag="ps", name=f"psk{sb}_{i}")
                        for i in range(HPC)]
                for eb in range(NB):
                    hst = hpool.tile([128, EB, 512], BF16, tag="h")
                    nc.sync.dma_start(hst, hsT_b[:, eb, :, ss])
                    for g in range(EB):
                        e = eb * EB + g
                        for jt in range(HPC):
                            js = slice(jt * 128, (jt + 1) * 128)
                            nc.tensor.matmul(
                                ps_q[jt], wq_m[:, e, js], hst[:, g],
                                start=(e == 0), stop=(e == KT - 1),
                            )
                            nc.tensor.matmul(
                                ps_k[jt], wk_m[:, e, js], hst[:, g],
                                start=(e == 0), stop=(e == KT - 1),
                            )
                # RoPE directly out of PSUM:
                #   dst = p*cos + swap(p)*shat   (swap halves; sign in shat)
                # RoPE: dst = qf*cos + rot_half(qf)*sin. shat holds the
                # HALF-SWAPPED signed sin (host-prepared), so both swap
                # multiplies have partition-ALIGNED inputs and only the
                # OUTPUT is partition-shifted (verified legal on HW).
                for ps_list, dst in ((ps_q, rq), (ps_k, rk)):
                    for jt in range(HPC):
                        p = ps_list[jt]
                        qf = rtmp.tile([128, 512], F32, tag="rt")
                        nc.scalar.copy(qf, p)   # frees the PSUM bank early
                        tmp = rtmp.tile([128, 512], F32, tag="rt")
                        nc.gpsimd.tensor_mul(
                            tmp[64:128], qf[0:64], shat_m[0:64, jt, ss]
                        )
                        nc.vector.tensor_mul(
                            tmp[0:64], qf[64:128], shat_m[64:128, jt, ss]
                        )
                        t2 = rtmp.tile([128, 512], F32, tag="rt")
                        nc.vector.tensor_mul(t2, qf, cos_m[:, jt, ss])
                        with nc.allow_low_precision(reason="rope bf16 store"):
                            nc.vector.tensor_add(dst[:, jt, ss], t2, tmp)

            # ---- phase 2: V projection (natural layout) ----
            wv_m = wres.tile([128, KT, JC], BF16, tag="w", name="wv")
            for c4 in range(4):
                cs = slice(c4 * 8, (c4 + 1) * 8)
                nc.sync.dma_start(wv_m[:, cs], wvT_b[:, cs])
            wo_m = wres.tile([128, HPC, HID], BF16, tag="w", name="wo")
            for c4 in range(4):
                cs = slice(c4 * 1024, (c4 + 1) * 1024)
                nc.sync.dma_start(wo_m[:, :, cs], woT_b[:, :, cs])

            v_m = big.tile([128, S // 128, JC], BF16, tag="big", name="v")
            for sb in range(SB):
                ss = slice(sb * 512, (sb + 1) * 512)
                ps_v = [ps.tile([128, 512], F32, tag="ps", name=f"psv{sb}_{i}")
                        for i in range(4)]
                for eb in range(NB):
                    hst = hpool.tile([128, EB, 512], BF16, tag="h")
                    nc.sync.dma_start(hst, hsT_b[:, eb, :, ss])
                    for g in range(EB):
                        e = eb * EB + g
                        for t4 in range(4):
                            cs = slice(t4 * 128, (t4 + 1) * 128)
                            nc.tensor.matmul(
                                ps_v[t4], hst[:, g, cs], wv_m[:, e],
                                start=(e == 0), stop=(e == KT - 1),
                            )
                for t4 in range(4):
                    nc.scalar.copy(v_m[:, sb * 4 + t4], ps_v[t4])

            # ---- phase 3+4: attention (qb outer) + interleaved o_proj ----
            attnT = big.tile([128, HPC * SB, 512], BF16, tag="big", name="attnT")
            oo_flip = 0
            for qb in range(SB):
                qs = slice(qb * 512, (qb + 1) * 512)
                nkt = 4 * qb + 4
                zrs = []
                pos_ = []
                for h in range(HPC):
                    i16 = h * SB + qb
                    ps_o = ps.tile([128, 512], F32, tag="ps", name=f"pso{qb}_{h}")
                    ps_z = ps.tile([1, 512], F32, tag="ps", name=f"psz{qb}_{h}")
                    for kt in range(nkt):
                        ps_s = ps.tile([128, 512], F32, tag="ps",
                                       name=f"pss{qb}_{h}_{kt}")
                        nc.tensor.matmul(
                            ps_s, rk[:, h, kt * 128: (kt + 1) * 128],
                            rq[:, h, qs], start=True, stop=True,
                        )
                        r = kt - 4 * qb
                        if r >= 0:
                            nc.vector.tensor_add(ps_s, ps_s, masks_sb[:, r])
                        ext = expp.tile([128, 512], BF16, tag="exp")
                        nc.scalar.activation(
                            ext, ps_s, mybir.ActivationFunctionType.Exp,
                            scale=SCALE,
                        )
                        nc.tensor.matmul(
                            ps_z, ones_bf, ext,
                            start=(kt == 0), stop=(kt == nkt - 1),
                        )
                        nc.tensor.matmul(
                            ps_o, v_m[:, kt, h * 128: (h + 1) * 128], ext,
                            start=(kt == 0), stop=(kt == nkt - 1),
                        )
                    # drain unnormalized rows (frees ps_o), 1/z row (frees ps_z)
                    nc.vector.tensor_copy(attnT[:, i16], ps_o)
                    zr = zp.tile([1, 512], F32R, tag="zr")
                    with nc.allow_low_precision(reason="1/z broadcast via PE"):
                        nc.vector.reciprocal(zr, ps_z)
                    zrs.append(zr)
                    pos_.append(i16)
                # normalize: broadcast 1/z across partitions on the PE
                for h in range(HPC):
                    zb = ps.tile([128, 512], F32, tag="ps", name=f"zb{qb}_{h}")
                    nc.tensor.matmul(zb, ones_row, zrs[h], start=True, stop=True)
                    i16 = pos_[h]
                    with nc.allow_low_precision(reason="attn normalize bf16"):
                        nc.vector.tensor_tensor(
                            attnT[:, i16], attnT[:, i16], zb,
                            op=mybir.AluOpType.mult,
                        )
                # o_proj partial columns for this query block
                for et in range(KT):
                    ps_oo = ps.tile([128, 512], F32, tag="ps",
                                    name=f"poo{qb}_{et}")
                    for jt in range(HPC):
                        nc.tensor.matmul(
                            ps_oo, wo_m[:, jt, et * 128: (et + 1) * 128],
                            attnT[:, jt * SB + qb],
                            start=(jt == 0), stop=(jt == HPC - 1),
                        )
                    oo = outp.tile([128, 512], F32, tag="oo")
                    if oo_flip % 2 == 0:
                        nc.scalar.copy(oo, ps_oo)
                    else:
                        nc.vector.tensor_copy(oo, ps_oo)
                    oo_flip += 1
                    nc.sync.dma_start(oT_b[:, et, qs], oo)

    nc.compile()
    return nc


def _get_nc():
    if "S" not in _CACHE:
        _CACHE["S"] = build()
    return _CACHE["S"]


def _causal_mask_templates():
    # masked (NEGM) iff 128*r + p > j for p in [0,128), j in [0,512)
    p = np.arange(128)[:, None]
    j = np.arange(512)[None, :]
    out = np.zeros((128, 4, 512), np.float32)
    for r in range(4):
        out[:, r, :] = np.where(128 * r + p > j, NEGM, 0.0).astype(np.float32)
    return np.ascontiguousarray(out.reshape(128, 4 * 512))


def _rope_cache_np():
    # mirrors reference._rope_cache in float32
    inv_freq = (1.0 / (BASE ** (np.arange(0, D, 2, dtype=np.float32) / np.float32(D)))).astype(np.float32)
    ratio = (MIN_R + (MAX_R - MIN_R) * (np.arange(H, dtype=np.float32) / np.float32(H))).astype(np.float32)
    t = (np.arange(S, dtype=np.float32)[None, :] / ratio[:, None]).astype(np.float32)
    freqs = (t[:, :, None] * inv_freq[None, None, :]).astype(np.float32)
    emb = np.concatenate([freqs, freqs], axis=-1)
    return np.cos(emb).astype(np.float32), np.sin(emb).astype(np.float32)


def _head_order(hs, Wq, Wk):
    """Exact head-outlier ordering from the last pre-RoPE attention row,
    computed in f64 on the host: srow_h = hs @ (Wk_h^T (Wq_h hs[-1]))."""
    hs64 = hs.astype(np.float64)
    q_last = hs64[-1] @ Wq.T.astype(np.float64)                 # [HID]
    Wk64 = Wk.astype(np.float64)
    Wall = np.empty((HID, H), np.float64)
    for h in range(H):
        rows = slice(h * D, (h + 1) * D)
        Wall[:, h] = Wk64[rows, :].T @ q_last[rows]
    srow = (hs64 @ Wall).T                                      # [H, S]
    sc = srow * SCALE
    m = sc.max(axis=-1, keepdims=True)
    e = np.exp(sc - m)
    aw = e / e.sum(axis=-1, keepdims=True)
    avg = aw.mean(axis=-1, keepdims=True)
    cnt = (aw > 3.0 * avg).sum(axis=-1)
    outlier = (-(cnt / np.float32(S))).astype(np.float32)
    return np.argsort(outlier, kind="stable")


def kernel(hidden_states, position_ids, Wq, Wk, Wv, Wo):
    hs = np.asarray(hidden_states, dtype=np.float32)[0]        # [S, HID]
    pos = np.asarray(position_ids).astype(np.int64)[0]         # [S]
    Wq = np.asarray(Wq, dtype=np.float32)
    Wk = np.asarray(Wk, dtype=np.float32)
    Wv = np.asarray(Wv, dtype=np.float32)
    Wo = np.asarray(Wo, dtype=np.float32)

    # ---- host: head order (exact control flow), permuted RoPE caches ----
    head_order = _head_order(hs, Wq, Wk)
    cos, sin = _rope_cache_np()
    cos_o = cos[head_order][:, pos, :]                         # [H, S, D]
    sin_o = sin[head_order][:, pos, :]
    masks = _causal_mask_templates()

    hsT = np.ascontiguousarray(hs.T).astype(NPBF16)            # [HID, S] bf16

    nc = _get_nc()
    in_maps = []
    for c in range(NCORES):
        rows = slice(c * JC, (c + 1) * JC)
        ct = np.ascontiguousarray(
            np.concatenate([cos_o[c * HPC + i].T for i in range(HPC)], axis=0)
        )  # [JC, S]
        # half-swapped signed sin: spre[0:64] = +sin[64:128],
        # spre[64:128] = -sin[0:64] (per head) — see RoPE comment in build()
        st = np.concatenate(
            [
                np.concatenate(
                    [sin_o[c * HPC + i].T[D // 2:], -sin_o[c * HPC + i].T[: D // 2]],
                    axis=0,
                )
                for i in range(HPC)
            ],
            axis=0,
        )
        in_maps.append(
            {
                "hsT": hsT,
                "wqT": np.ascontiguousarray(Wq[rows, :].T).astype(NPBF16),
                "wkT": np.ascontiguousarray(Wk[rows, :].T).astype(NPBF16),
                "wvT": np.ascontiguousarray(Wv[rows, :].T).astype(NPBF16),
                "woT": np.ascontiguousarray(Wo[:, rows].T).astype(NPBF16),
                "cosT": ct.astype(NPBF16),
                "shatT": np.ascontiguousarray(st).astype(NPBF16),
                "masks": masks,
            }
        )
    res = bass_utils.run_bass_kernel_spmd(
        nc, in_maps, core_ids=list(range(NCORES)), trace=TRACE
    )
    if TRACE:
        LAST_PROFILE["S"] = res

    # ---- host: unshard (sum o_proj partials) ----
    acc = np.zeros((HID, S), np.float64)
    for c in range(NCORES):
        acc += res.results[c]["oT"]
    return np.ascontiguousarray(acc.T)[None, :, :].astype(np.float32)


# revision 10
# speedup vs baseline: 1.4143x; 1.0930x over previous
"""MsPoE Llama attention on 8 TRN2 NeuronCores (tensor-parallel over heads).

Strategy (v2: single launch)
----------------------------
The reference's head-ordering statistic only needs the LAST pre-RoPE
attention row: srow_h = q_last_h . k_h[s]. By associativity,
srow_h = hs @ (Wk_h^T (Wq_h hs[-1])) — ~0.5 GFLOP, computed on the
host in float64 BEFORE launching (verified to reproduce the reference
head_order exactly: min margin to the 3*avg threshold is ~1e-5 vs
~6e-7 fp32-path noise). With head_order known up-front, the permuted
per-head RoPE cos/sin caches become plain inputs and the whole module
runs in ONE device launch with q/k/v resident in SBUF:

  Per core (4 heads), all matmul operands bf16 (1 PE cycle/row — the
  same rate as fp32r — but half the DMA/SBUF, and no f32->f32r
  conversion casts at all):

  1. QK pass: stream hsT once, accumulate q/k head-blocks in PSUM
     (8 banks), apply RoPE directly out of PSUM on DVE (+gpsimd for
     one swap-half) into resident rq/rk [128, 4, 2048] bf16 tiles.
     rotate_half's partition swap is two half-partition reads at
     offset 64/0; the sin sign flip is pre-applied on the host (shat).
  2. V pass: stream hsT again (wq+wk+wv cannot be resident at once),
     natural-layout V into resident v_m [128, 16, 512] bf16.
  3. Attention per (qb outer, head inner): scores^T = rk_chunk^T rq
     per 128-key tile, exp on ACT (bf16 out), softmax denominator via
     ones-matmul accumulation, unnormalized AV accumulation; 1/z is
     partition-broadcast with a tiny PE outer-product (no DRAM
     roundtrip); attnT normalized in place (DVE).
  4. o_proj for qb is emitted right after qb's heads finish, so its
     matmuls overlap the next qb's attention chain; partials oT
     [4096, 2048] f32 stream out per 512-column block.

  Host: sum the 8 o_proj partials (f64) -> [1, 2048, 4096].
"""

import os
import sys

import numpy as np

for _p in ("/opt/trn_rl_repo", "/root/.axon_site/_ro/trn_rl_repo"):
    if os.path.isdir(_p) and _p not in sys.path:
        sys.path.append(_p)

import concourse.bass as bass  # noqa: E402
import concourse.tile as tile  # noqa: E402
from concourse import bacc, mybir  # noqa: E402
from concourse import bass_utils  # noqa: E402

import ml_dtypes  # noqa: E402

F32 = mybir.dt.float32
BF16 = mybir.dt.bfloat16
F32R = mybir.dt.float32r
NPBF16 = ml_dtypes.bfloat16

B, S, HID, H, D = 1, 2048, 4096, 32, 128
NCORES, HPC = 8, 4          # cores, heads per core
JC = HPC * D                # 512: per-core projection width
KT = HID // 128             # 32 contraction tiles
SB = S // 512               # 4 sequence blocks
EB = 4                      # e-tiles per hs-stream DMA
BASE, MIN_R, MAX_R = 10000.0, 1.0, 3.0
SCALE = 1.0 / float(np.sqrt(D))
NEGM = -1.0e35              # additive causal mask value (exp -> 0)

_CACHE = {}
TRACE = False          # set True (e.g. from test.py) to profile the launch
LAST_PROFILE = {}      # filled with BassKernelResults when TRACE is on


def build():
    nc = bacc.Bacc("TRN2", target_bir_lowering=False, debug=False, num_devices=NCORES)
    hsT = nc.dram_tensor("hsT", [HID, S], BF16, kind="ExternalInput").ap()
    wqT = nc.dram_tensor("wqT", [HID, JC], BF16, kind="ExternalInput").ap()
    wkT = nc.dram_tensor("wkT", [HID, JC], BF16, kind="ExternalInput").ap()
    wvT = nc.dram_tensor("wvT", [HID, JC], BF16, kind="ExternalInput").ap()
    woT = nc.dram_tensor("woT", [JC, HID], BF16, kind="ExternalInput").ap()
    cosT = nc.dram_tensor("cosT", [JC, S], BF16, kind="ExternalInput").ap()
    shatT = nc.dram_tensor("shatT", [JC, S], BF16, kind="ExternalInput").ap()
    masks = nc.dram_tensor("masks", [128, 4 * 512], F32, kind="ExternalInput").ap()
    oT = nc.dram_tensor("oT", [HID, S], F32, kind="ExternalOutput").ap()

    hsT_b = hsT.rearrange("(eb g p) s -> p eb g s", p=128, g=EB)   # [128, 8, EB, S]
    wqT_b = wqT.rearrange("(eb g p) j -> p eb g j", p=128, g=EB)   # [128, 8, EB, JC]
    wkT_b = wkT.rearrange("(eb g p) j -> p eb g j", p=128, g=EB)
    wvT_b = wvT.rearrange("(kt p) j -> p kt j", p=128)             # [128, 32, JC]
    woT_b = woT.rearrange("(jt p) e -> p jt e", p=128)             # [128, 4, HID]
    cosT_b = cosT.rearrange("(h p) s -> p h s", p=128)             # [128, 4, S]
    shatT_b = shatT.rearrange("(h p) s -> p h s", p=128)
    oT_b = oT.rearrange("(et p) s -> p et s", p=128)               # [128, 32, S]
    NB = KT // EB

    with tile.TileContext(nc) as tc:
        with (
            tc.tile_pool(name="wres", bufs=2) as wres,       # wv_m, wo_m resident
            tc.tile_pool(name="wst", bufs=4) as wst,         # wq/wk streamed chunks
            tc.tile_pool(name="trig", bufs=4) as trig,       # cos/shat per-sb stream
            tc.tile_pool(name="big", bufs=4) as big,         # rq/rk/v/attnT resident
            tc.tile_pool(name="hpool", bufs=2) as hpool,     # hs stream tiles
            tc.tile_pool(name="rtmp", bufs=4) as rtmp,       # RoPE f32 temps
            tc.tile_pool(name="expp", bufs=3) as expp,       # exp tiles bf16
            tc.tile_pool(name="small", bufs=1) as small,
            tc.tile_pool(name="zp", bufs=4) as zp,           # 1/z rows
            tc.tile_pool(name="outp", bufs=3) as outp,
            tc.tile_pool(name="ps", bufs=8, space="PSUM") as ps,
        ):
            rq = big.tile([128, HPC, S], BF16, tag="big", name="rq")
            rk = big.tile([128, HPC, S], BF16, tag="big", name="rk")
            wv_m = wres.tile([128, KT, JC], BF16, tag="w", name="wv")
            wo_m = wres.tile([128, HPC, HID], BF16, tag="w", name="wo")

            # ---- phase 1: QK projections + fused RoPE ----
            # wq/wk stream in eb-sized chunks alongside hs, so the PE can
            # start ~3 DMAs in; wv/wo resident loads trickle in one chunk
            # per sb and are ready by phase 2.
            for sb in range(SB):
                ss = slice(sb * 512, (sb + 1) * 512)
                ps_q = [ps.tile([128, 512], F32, tag="ps", name=f"psq{sb}_{i}")
                        for i in range(HPC)]
                ps_k = [ps.tile([128, 512], F32, tag="ps", name=f"psk{sb}_{i}")
                        for i in range(HPC)]
                cos_t = trig.tile([128, HPC, 512], BF16, tag="t", name=f"cos{sb}")
                shat_t = trig.tile([128, HPC, 512], BF16, tag="t", name=f"shat{sb}")
                for eb in range(NB):
                    hst = hpool.tile([128, EB, 512], BF16, tag="h")
                    nc.sync.dma_start(hst, hsT_b[:, eb, :, ss])
                    wqs = wst.tile([128, EB, JC], BF16, tag="w", name=f"wqs{sb}_{eb}")
                    nc.sync.dma_start(wqs, wqT_b[:, eb])
                    wks = wst.tile([128, EB, JC], BF16, tag="w", name=f"wks{sb}_{eb}")
                    nc.sync.dma_start(wks, wkT_b[:, eb])
                    if eb == 0:
                        nc.sync.dma_start(cos_t, cosT_b[:, :, ss])
                        nc.sync.dma_start(shat_t, shatT_b[:, :, ss])
                    for g in range(EB):
                        e = eb * EB + g
                        for jt in range(HPC):
                            js = slice(jt * 128, (jt + 1) * 128)
                            nc.tensor.matmul(
                                ps_q[jt], wqs[:, g, js], hst[:, g],
                                start=(e == 0), stop=(e == KT - 1),
                            )
                            nc.tensor.matmul(
                                ps_k[jt], wks[:, g, js], hst[:, g],
                                start=(e == 0), stop=(e == KT - 1),
                            )
                # prefetch one quarter of wv/wo per sb
                pcs = slice(sb * 8, (sb + 1) * 8)
                nc.sync.dma_start(wv_m[:, pcs], wvT_b[:, pcs])
                nc.sync.dma_start(wo_m[:, :, sb * 1024:(sb + 1) * 1024],
                                  woT_b[:, :, sb * 1024:(sb + 1) * 1024])
                # RoPE: dst = qf*cos + rot_half(qf)*sin. shat holds the
                # HALF-SWAPPED signed sin (host-prepared), so both swap
                # multiplies have partition-ALIGNED inputs and only the
                # OUTPUT is partition-shifted (verified exact on HW).
                for ps_list, dst in ((ps_q, rq), (ps_k, rk)):
                    for jt in range(HPC):
                        p = ps_list[jt]
                        qf = rtmp.tile([128, 512], F32, tag="rt")
                        nc.scalar.copy(qf, p)   # frees the PSUM bank early
                        tmp = rtmp.tile([128, 512], F32, tag="rt")
                        nc.gpsimd.tensor_mul(
                            tmp[64:128], qf[0:64], shat_t[0:64, jt]
                        )
                        nc.vector.tensor_mul(
                            tmp[0:64], qf[64:128], shat_t[64:128, jt]
                        )
                        t2 = rtmp.tile([128, 512], F32, tag="rt")
                        nc.vector.tensor_mul(t2, qf, cos_t[:, jt])
                        with nc.allow_low_precision(reason="rope bf16 store"):
                            nc.vector.tensor_add(dst[:, jt, ss], t2, tmp)

            # ---- constants for attention (loaded behind phase 2's stream) ----
            masks_sb = small.tile([128, 4, 512], F32)
            nc.sync.dma_start(masks_sb, masks.rearrange("p (r j) -> p r j", r=4))
            onesf = small.tile([128, 1], F32)
            nc.vector.memset(onesf, 1.0)
            ones_bf = small.tile([128, 1], BF16)
            nc.vector.tensor_copy(ones_bf, onesf)
            onesf_r = small.tile([1, 128], F32)
            nc.vector.memset(onesf_r, 1.0)
            ones_row = small.tile([1, 128], F32R)
            nc.vector.tensor_copy(ones_row, onesf_r)

            # ---- phase 2: V projection (natural layout) ----
            v_m = big.tile([128, S // 128, JC], BF16, tag="big", name="v")
            for sb in range(SB):
                ss = slice(sb * 512, (sb + 1) * 512)
                ps_v = [ps.tile([128, 512], F32, tag="ps", name=f"psv{sb}_{i}")
                        for i in range(4)]
                for eb in range(NB):
                    hst = hpool.tile([128, EB, 512], BF16, tag="h")
                    nc.sync.dma_start(hst, hsT_b[:, eb, :, ss])
                    for g in range(EB):
                        e = eb * EB + g
                        for t4 in range(4):
                            cs = slice(t4 * 128, (t4 + 1) * 128)
                            nc.tensor.matmul(
                                ps_v[t4], hst[:, g, cs], wv_m[:, e],
                                start=(e == 0), stop=(e == KT - 1),
                            )
                for t4 in range(4):
                    nc.scalar.copy(v_m[:, sb * 4 + t4], ps_v[t4])

            # ---- phase 3+4: attention (qb outer) + interleaved o_proj ----
            attnT = big.tile([128, HPC * SB, 512], BF16, tag="big", name="attnT")
            oo_flip = 0
            for qb in range(SB):
                qs = slice(qb * 512, (qb + 1) * 512)
                nkt = 4 * qb + 4
                zrs = []
                pos_ = []
                for h in range(HPC):
                    i16 = h * SB + qb
                    ps_o = ps.tile([128, 512], F32, tag="ps", name=f"pso{qb}_{h}")
                    ps_z = ps.tile([1, 512], F32, tag="ps", name=f"psz{qb}_{h}")
                    for kt in range(nkt):
                        ps_s = ps.tile([128, 512], F32, tag="ps",
                                       name=f"pss{qb}_{h}_{kt}")
                        nc.tensor.matmul(
                            ps_s, rk[:, h, kt * 128: (kt + 1) * 128],
                            rq[:, h, qs], start=True, stop=True,
                        )
                        r = kt - 4 * qb
                        if r >= 0:
                            nc.vector.tensor_add(ps_s, ps_s, masks_sb[:, r])
                        ext = expp.tile([128, 512], BF16, tag="exp")
                        nc.scalar.activation(
                            ext, ps_s, mybir.ActivationFunctionType.Exp,
                            scale=SCALE,
                        )
                        nc.tensor.matmul(
                            ps_z, ones_bf, ext,
                            start=(kt == 0), stop=(kt == nkt - 1),
                        )
                        nc.tensor.matmul(
                            ps_o, v_m[:, kt, h * 128: (h + 1) * 128], ext,
                            start=(kt == 0), stop=(kt == nkt - 1),
                        )
                    # drain unnormalized rows (frees ps_o), 1/z row (frees ps_z)
                    nc.vector.tensor_copy(attnT[:, i16], ps_o)
                    zf = zp.tile([1, 512], F32, tag="zf", bufs=2)
                    nc.vector.reciprocal_approx_fast(zf, ps_z)
                    zr = zp.tile([1, 512], F32R, tag="zr")
                    nc.vector.tensor_copy(zr, zf)
                    zrs.append(zr)
                    pos_.append(i16)
                # normalize: broadcast 1/z across partitions on the PE
                for h in range(HPC):
                    zb = ps.tile([128, 512], F32, tag="ps", name=f"zb{qb}_{h}")
                    nc.tensor.matmul(zb, ones_row, zrs[h], start=True, stop=True)
                    i16 = pos_[h]
                    with nc.allow_low_precision(reason="attn normalize bf16"):
                        nc.vector.tensor_tensor(
                            attnT[:, i16], attnT[:, i16], zb,
                            op=mybir.AluOpType.mult,
                        )
                # o_proj partial columns for this query block
                for et in range(KT):
                    ps_oo = ps.tile([128, 512], F32, tag="ps",
                                    name=f"poo{qb}_{et}")
                    for jt in range(HPC):
                        nc.tensor.matmul(
                            ps_oo, wo_m[:, jt, et * 128: (et + 1) * 128],
                            attnT[:, jt * SB + qb],
                            start=(jt == 0), stop=(jt == HPC - 1),
                        )
                    oo = outp.tile([128, 512], F32, tag="oo")
                    if oo_flip % 2 == 0:
                        nc.scalar.copy(oo, ps_oo)
                    else:
                        nc.vector.tensor_copy(oo, ps_oo)
                    oo_flip += 1
                    nc.sync.dma_start(oT_b[:, et, qs], oo)

    nc.compile()
    return nc


def _get_nc():
    if "S" not in _CACHE:
        _CACHE["S"] = build()
    return _CACHE["S"]


def _causal_mask_templates():
    # masked (NEGM) iff 128*r + p > j for p in [0,128), j in [0,512)
    p = np.arange(128)[:, None]
    j = np.arange(512)[None, :]
    out = np.zeros((128, 4, 512), np.float32)
    for r in range(4):
        out[:, r, :] = np.where(128 * r + p > j, NEGM, 0.0).astype(np.float32)
    return np.ascontiguousarray(out.reshape(128, 4 * 512))


def _rope_cache_np():
    # mirrors reference._rope_cache in float32
    inv_freq = (1.0 / (BASE ** (np.arange(0, D, 2, dtype=np.float32) / np.float32(D)))).astype(np.float32)
    ratio = (MIN_R + (MAX_R - MIN_R) * (np.arange(H, dtype=np.float32) / np.float32(H))).astype(np.float32)
    t = (np.arange(S, dtype=np.float32)[None, :] / ratio[:, None]).astype(np.float32)
    freqs = (t[:, :, None] * inv_freq[None, None, :]).astype(np.float32)
    emb = np.concatenate([freqs, freqs], axis=-1)
    return np.cos(emb).astype(np.float32), np.sin(emb).astype(np.float32)


def _head_order(hs, Wq, Wk):
    """Exact head-outlier ordering from the last pre-RoPE attention row,
    computed in f64 on the host: srow_h = hs @ (Wk_h^T (Wq_h hs[-1]))."""
    hs64 = hs.astype(np.float64)
    q_last = hs64[-1] @ Wq.T.astype(np.float64)                 # [HID]
    Wk64 = Wk.astype(np.float64)
    Wall = np.empty((HID, H), np.float64)
    for h in range(H):
        rows = slice(h * D, (h + 1) * D)
        Wall[:, h] = Wk64[rows, :].T @ q_last[rows]
    srow = (hs64 @ Wall).T                                      # [H, S]
    sc = srow * SCALE
    m = sc.max(axis=-1, keepdims=True)
    e = np.exp(sc - m)
    aw = e / e.sum(axis=-1, keepdims=True)
    avg = aw.mean(axis=-1, keepdims=True)
    cnt = (aw > 3.0 * avg).sum(axis=-1)
    outlier = (-(cnt / np.float32(S))).astype(np.float32)
    return np.argsort(outlier, kind="stable")


def kernel(hidden_states, position_ids, Wq, Wk, Wv, Wo):
    hs = np.asarray(hidden_states, dtype=np.float32)[0]        # [S, HID]
    pos = np.asarray(position_ids).astype(np.int64)[0]         # [S]
    Wq = np.asarray(Wq, dtype=np.float32)
    Wk = np.asarray(Wk, dtype=np.float32)
    Wv = np.asarray(Wv, dtype=np.float32)
    Wo = np.asarray(Wo, dtype=np.float32)

    # ---- host: head order (exact control flow), permuted RoPE caches ----
    head_order = _head_order(hs, Wq, Wk)
    cos, sin = _rope_cache_np()
    cos_o = cos[head_order][:, pos, :]                         # [H, S, D]
    sin_o = sin[head_order][:, pos, :]
    masks = _causal_mask_templates()

    hsT = np.ascontiguousarray(hs.T).astype(NPBF16)            # [HID, S] bf16

    nc = _get_nc()
    in_maps = []
    for c in range(NCORES):
        rows = slice(c * JC, (c + 1) * JC)
        ct = np.ascontiguousarray(
            np.concatenate([cos_o[c * HPC + i].T for i in range(HPC)], axis=0)
        )  # [JC, S]
        # half-swapped signed sin: spre[0:64] = +sin[64:128],
        # spre[64:128] = -sin[0:64] (per head) — see RoPE comment in build()
        st = np.concatenate(
            [
                np.concatenate(
                    [sin_o[c * HPC + i].T[D // 2:], -sin_o[c * HPC + i].T[: D // 2]],
                    axis=0,
                )
                for i in range(HPC)
            ],
            axis=0,
        )
        in_maps.append(
            {
                "hsT": hsT,
                "wqT": np.ascontiguousarray(Wq[rows, :].T).astype(NPBF16),
                "wkT": np.ascontiguousarray(Wk[rows, :].T).astype(NPBF16),
                "wvT": np.ascontiguousarray(Wv[rows, :].T).astype(NPBF16),
                "woT": np.ascontiguousarray(Wo[:, rows].T).astype(NPBF16),
                "cosT": ct.astype(NPBF16),
                "shatT": np.ascontiguousarray(st).astype(NPBF16),
                "masks": masks,
            }
        )
    res = bass_utils.run_bass_kernel_spmd(
        nc, in_maps, core_ids=list(range(NCORES)), trace=TRACE
    )
    if TRACE:
        LAST_PROFILE["S"] = res

    # ---- host: unshard (sum o_proj partials) ----
    acc = np.zeros((HID, S), np.float64)
    for c in range(NCORES):
        acc += res.results[c]["oT"]
    return np.ascontiguousarray(acc.T)[None, :, :].astype(np.float32)


# revision 11
# speedup vs baseline: 1.5537x; 1.0986x over previous
"""MsPoE Llama attention on 8 TRN2 NeuronCores (tensor-parallel over heads).

Strategy (v2: single launch)
----------------------------
The reference's head-ordering statistic only needs the LAST pre-RoPE
attention row: srow_h = q_last_h . k_h[s]. By associativity,
srow_h = hs @ (Wk_h^T (Wq_h hs[-1])) — ~0.5 GFLOP, computed on the
host in float64 BEFORE launching (verified to reproduce the reference
head_order exactly: min margin to the 3*avg threshold is ~1e-5 vs
~6e-7 fp32-path noise). With head_order known up-front, the permuted
per-head RoPE cos/sin caches become plain inputs and the whole module
runs in ONE device launch with q/k/v resident in SBUF:

  Per core (4 heads), all matmul operands bf16 (1 PE cycle/row — the
  same rate as fp32r — but half the DMA/SBUF, and no f32->f32r
  conversion casts at all):

  1. QK pass: stream hsT once, accumulate q/k head-blocks in PSUM
     (8 banks), apply RoPE directly out of PSUM on DVE (+gpsimd for
     one swap-half) into resident rq/rk [128, 4, 2048] bf16 tiles.
     rotate_half's partition swap is two half-partition reads at
     offset 64/0; the sin sign flip is pre-applied on the host (shat).
  2. V pass: stream hsT again (wq+wk+wv cannot be resident at once),
     natural-layout V into resident v_m [128, 16, 512] bf16.
  3. Attention per (qb outer, head inner): scores^T = rk_chunk^T rq
     per 128-key tile, exp on ACT (bf16 out), softmax denominator via
     ones-matmul accumulation, unnormalized AV accumulation; 1/z is
     partition-broadcast with a tiny PE outer-product (no DRAM
     roundtrip); attnT normalized in place (DVE).
  4. o_proj for qb is emitted right after qb's heads finish, so its
     matmuls overlap the next qb's attention chain; partials oT
     [4096, 2048] f32 stream out per 512-column block.

  Host: sum the 8 o_proj partials (f64) -> [1, 2048, 4096].
"""

import os
import sys

import numpy as np

for _p in ("/opt/trn_rl_repo", "/root/.axon_site/_ro/trn_rl_repo"):
    if os.path.isdir(_p) and _p not in sys.path:
        sys.path.append(_p)

import concourse.bass as bass  # noqa: E402
import concourse.tile as tile  # noqa: E402
from concourse import bacc, mybir  # noqa: E402
from concourse import bass_utils  # noqa: E402

import ml_dtypes  # noqa: E402

F32 = mybir.dt.float32
BF16 = mybir.dt.bfloat16
F32R = mybir.dt.float32r
NPBF16 = ml_dtypes.bfloat16

B, S, HID, H, D = 1, 2048, 4096, 32, 128
NCORES, HPC = 8, 4          # cores, heads per core
JC = HPC * D                # 512: per-core projection width
KT = HID // 128             # 32 contraction tiles
SB = S // 512               # 4 sequence blocks
EB = 4                      # e-tiles per hs-stream DMA
BASE, MIN_R, MAX_R = 10000.0, 1.0, 3.0
SCALE = 1.0 / float(np.sqrt(D))
NEGM = -1.0e35              # additive causal mask value (exp -> 0)

_CACHE = {}
TRACE = False          # set True (e.g. from test.py) to profile the launch
LAST_PROFILE = {}      # filled with BassKernelResults when TRACE is on


def build():
    nc = bacc.Bacc("TRN2", target_bir_lowering=False, debug=False, num_devices=NCORES)
    hsT = nc.dram_tensor("hsT", [HID, S], BF16, kind="ExternalInput").ap()
    wqT = nc.dram_tensor("wqT", [HID, JC], BF16, kind="ExternalInput").ap()
    wkT = nc.dram_tensor("wkT", [HID, JC], BF16, kind="ExternalInput").ap()
    wvT = nc.dram_tensor("wvT", [HID, JC], BF16, kind="ExternalInput").ap()
    woT = nc.dram_tensor("woT", [JC, HID], BF16, kind="ExternalInput").ap()
    cosT = nc.dram_tensor("cosT", [JC, S], BF16, kind="ExternalInput").ap()
    shatT = nc.dram_tensor("shatT", [JC, S], BF16, kind="ExternalInput").ap()
    masks = nc.dram_tensor("masks", [128, 4 * 512], F32, kind="ExternalInput").ap()
    oT = nc.dram_tensor("oT", [HID, S], BF16, kind="ExternalOutput").ap()

    hsT_b = hsT.rearrange("(eb g p) s -> p eb g s", p=128, g=EB)   # [128, 8, EB, S]
    wqT_b = wqT.rearrange("(eb g p) j -> p eb g j", p=128, g=EB)   # [128, 8, EB, JC]
    wkT_b = wkT.rearrange("(eb g p) j -> p eb g j", p=128, g=EB)
    wvT_b = wvT.rearrange("(kt p) j -> p kt j", p=128)             # [128, 32, JC]
    woT_b = woT.rearrange("(jt p) e -> p jt e", p=128)             # [128, 4, HID]
    cosT_b = cosT.rearrange("(h p) s -> p h s", p=128)             # [128, 4, S]
    shatT_b = shatT.rearrange("(h p) s -> p h s", p=128)
    oT_b = oT.rearrange("(et p) s -> p et s", p=128)               # [128, 32, S]
    NB = KT // EB

    with tile.TileContext(nc) as tc:
        with (
            tc.tile_pool(name="wres", bufs=2) as wres,       # wv_m, wo_m resident
            tc.tile_pool(name="wst", bufs=4) as wst,         # wq/wk streamed chunks
            tc.tile_pool(name="trig", bufs=4) as trig,       # cos/shat per-sb stream
            tc.tile_pool(name="big", bufs=4) as big,         # rq/rk/v/attnT resident
            tc.tile_pool(name="hpool", bufs=2) as hpool,     # hs stream tiles
            tc.tile_pool(name="rtmp", bufs=4) as rtmp,       # RoPE f32 temps
            tc.tile_pool(name="expp", bufs=3) as expp,       # exp tiles bf16
            tc.tile_pool(name="small", bufs=1) as small,
            tc.tile_pool(name="zp", bufs=4) as zp,           # 1/z rows
            tc.tile_pool(name="outp", bufs=4) as outp,
            tc.tile_pool(name="ps", bufs=8, space="PSUM") as ps,
        ):
            rq = big.tile([128, HPC, S], BF16, tag="big", name="rq")
            rk = big.tile([128, HPC, S], BF16, tag="big", name="rk")
            wv_m = wres.tile([128, KT, JC], BF16, tag="w", name="wv")
            wo_m = wres.tile([128, HPC, HID], BF16, tag="w", name="wo")

            # ---- phase 1: QK projections + fused RoPE ----
            # wq/wk stream in eb-sized chunks alongside hs, so the PE can
            # start ~3 DMAs in; wv/wo resident loads trickle in one chunk
            # per sb and are ready by phase 2.
            for sb in range(SB):
                ss = slice(sb * 512, (sb + 1) * 512)
                ps_q = [ps.tile([128, 512], F32, tag="ps", name=f"psq{sb}_{i}")
                        for i in range(HPC)]
                ps_k = [ps.tile([128, 512], F32, tag="ps", name=f"psk{sb}_{i}")
                        for i in range(HPC)]
                cos_t = trig.tile([128, HPC, 512], BF16, tag="t", name=f"cos{sb}")
                shat_t = trig.tile([128, HPC, 512], BF16, tag="t", name=f"shat{sb}")
                for eb in range(NB):
                    hst = hpool.tile([128, EB, 512], BF16, tag="h")
                    nc.sync.dma_start(hst, hsT_b[:, eb, :, ss])
                    wqs = wst.tile([128, EB, JC], BF16, tag="w", name=f"wqs{sb}_{eb}")
                    nc.sync.dma_start(wqs, wqT_b[:, eb])
                    wks = wst.tile([128, EB, JC], BF16, tag="w", name=f"wks{sb}_{eb}")
                    nc.sync.dma_start(wks, wkT_b[:, eb])
                    if eb == 0:
                        nc.sync.dma_start(cos_t, cosT_b[:, :, ss])
                        nc.sync.dma_start(shat_t, shatT_b[:, :, ss])
                    for g in range(EB):
                        e = eb * EB + g
                        for jt in range(HPC):
                            js = slice(jt * 128, (jt + 1) * 128)
                            nc.tensor.matmul(
                                ps_q[jt], wqs[:, g, js], hst[:, g],
                                start=(e == 0), stop=(e == KT - 1),
                            )
                            nc.tensor.matmul(
                                ps_k[jt], wks[:, g, js], hst[:, g],
                                start=(e == 0), stop=(e == KT - 1),
                            )
                # prefetch one quarter of wv/wo per sb
                pcs = slice(sb * 8, (sb + 1) * 8)
                nc.sync.dma_start(wv_m[:, pcs], wvT_b[:, pcs])
                nc.sync.dma_start(wo_m[:, :, sb * 1024:(sb + 1) * 1024],
                                  woT_b[:, :, sb * 1024:(sb + 1) * 1024])
                # RoPE: dst = qf*cos + rot_half(qf)*sin. shat holds the
                # HALF-SWAPPED signed sin (host-prepared), so both swap
                # multiplies have partition-ALIGNED inputs and only the
                # OUTPUT is partition-shifted (verified exact on HW).
                for ps_list, dst in ((ps_q, rq), (ps_k, rk)):
                    for jt in range(HPC):
                        p = ps_list[jt]
                        qf = rtmp.tile([128, 512], F32, tag="rt")
                        nc.scalar.copy(qf, p)   # frees the PSUM bank early
                        tmp = rtmp.tile([128, 512], F32, tag="rt")
                        nc.gpsimd.tensor_mul(
                            tmp[64:128], qf[0:64], shat_t[0:64, jt]
                        )
                        nc.vector.tensor_mul(
                            tmp[0:64], qf[64:128], shat_t[64:128, jt]
                        )
                        t2 = rtmp.tile([128, 512], F32, tag="rt")
                        nc.vector.tensor_mul(t2, qf, cos_t[:, jt])
                        with nc.allow_low_precision(reason="rope bf16 store"):
                            nc.vector.tensor_add(dst[:, jt, ss], t2, tmp)

            # ---- constants for attention (loaded behind phase 2's stream) ----
            masks_sb = small.tile([128, 4, 512], F32)
            nc.sync.dma_start(masks_sb, masks.rearrange("p (r j) -> p r j", r=4))
            onesf = small.tile([128, 1], F32)
            nc.vector.memset(onesf, 1.0)
            ones_bf = small.tile([128, 1], BF16)
            nc.vector.tensor_copy(ones_bf, onesf)
            onesf_r = small.tile([1, 128], F32)
            nc.vector.memset(onesf_r, 1.0)
            ones_row = small.tile([1, 128], F32R)
            nc.vector.tensor_copy(ones_row, onesf_r)

            # ---- phase 2: V projection (natural layout) ----
            v_m = big.tile([128, S // 128, JC], BF16, tag="big", name="v")
            for sb in range(SB):
                ss = slice(sb * 512, (sb + 1) * 512)
                ps_v = [ps.tile([128, 512], F32, tag="ps", name=f"psv{sb}_{i}")
                        for i in range(4)]
                for eb in range(NB):
                    hst = hpool.tile([128, EB, 512], BF16, tag="h")
                    nc.sync.dma_start(hst, hsT_b[:, eb, :, ss])
                    for g in range(EB):
                        e = eb * EB + g
                        for t4 in range(4):
                            cs = slice(t4 * 128, (t4 + 1) * 128)
                            nc.tensor.matmul(
                                ps_v[t4], hst[:, g, cs], wv_m[:, e],
                                start=(e == 0), stop=(e == KT - 1),
                            )
                for t4 in range(4):
                    nc.scalar.copy(v_m[:, sb * 4 + t4], ps_v[t4])

            # ---- phase 3+4: attention (qb outer) + interleaved o_proj ----
            attnT = big.tile([128, HPC * SB, 512], BF16, tag="big", name="attnT")
            oo_flip = 0
            for qb in range(SB):
                qs = slice(qb * 512, (qb + 1) * 512)
                nkt = 4 * qb + 4
                zrs = []
                pos_ = []
                for h in range(HPC):
                    i16 = h * SB + qb
                    ps_o = ps.tile([128, 512], F32, tag="ps", name=f"pso{qb}_{h}")
                    ps_z = ps.tile([1, 512], F32, tag="ps", name=f"psz{qb}_{h}")
                    for kt in range(nkt):
                        # diagonal blocks: columns j < 128*r are fully
                        # masked -> skip them (w = valid width)
                        r = kt - 4 * qb
                        j0 = 128 * r if r > 0 else 0
                        w = 512 - j0
                        qsw_ = slice(qb * 512 + j0, (qb + 1) * 512)
                        ps_s = ps.tile([128, w], F32, tag="ps",
                                       name=f"pss{qb}_{h}_{kt}")
                        nc.tensor.matmul(
                            ps_s, rk[:, h, kt * 128: (kt + 1) * 128],
                            rq[:, h, qsw_], start=True, stop=True,
                        )
                        if r >= 0:
                            nc.vector.tensor_add(
                                ps_s, ps_s, masks_sb[:, r, j0:512]
                            )
                        ext = expp.tile([128, w], BF16, tag="exp")
                        nc.scalar.activation(
                            ext, ps_s, mybir.ActivationFunctionType.Exp,
                            scale=SCALE,
                        )
                        nc.tensor.matmul(
                            ps_z[:, j0:512], ones_bf, ext,
                            start=(kt == 0), stop=(kt == nkt - 1),
                        )
                        nc.tensor.matmul(
                            ps_o[:, j0:512],
                            v_m[:, kt, h * 128: (h + 1) * 128], ext,
                            start=(kt == 0), stop=(kt == nkt - 1),
                        )
                    # drain unnormalized rows (frees ps_o), 1/z row (frees ps_z)
                    nc.vector.tensor_copy(attnT[:, i16], ps_o)
                    zf = zp.tile([1, 512], F32, tag="zf", bufs=2)
                    nc.vector.reciprocal_approx_fast(zf, ps_z)
                    zr = zp.tile([1, 512], F32R, tag="zr")
                    nc.vector.tensor_copy(zr, zf)
                    zrs.append(zr)
                    pos_.append(i16)
                # normalize: broadcast 1/z across partitions on the PE
                for h in range(HPC):
                    zb = ps.tile([128, 512], F32, tag="ps", name=f"zb{qb}_{h}")
                    nc.tensor.matmul(zb, ones_row, zrs[h], start=True, stop=True)
                    i16 = pos_[h]
                    with nc.allow_low_precision(reason="attn normalize bf16"):
                        nc.vector.tensor_tensor(
                            attnT[:, i16], attnT[:, i16], zb,
                            op=mybir.AluOpType.mult,
                        )
                # o_proj partial columns for this query block
                for et in range(KT):
                    ps_oo = ps.tile([128, 512], F32, tag="ps",
                                    name=f"poo{qb}_{et}")
                    for jt in range(HPC):
                        nc.tensor.matmul(
                            ps_oo, wo_m[:, jt, et * 128: (et + 1) * 128],
                            attnT[:, jt * SB + qb],
                            start=(jt == 0), stop=(jt == HPC - 1),
                        )
                    oo = outp.tile([128, 512], BF16, tag="oo", bufs=5)
                    if oo_flip % 2 == 0:
                        nc.scalar.copy(oo, ps_oo)
                    else:
                        nc.vector.tensor_copy(oo, ps_oo)
                    oo_flip += 1
                    nc.sync.dma_start(oT_b[:, et, qs], oo)

    nc.compile()
    return nc


def _get_nc():
    if "S" not in _CACHE:
        _CACHE["S"] = build()
    return _CACHE["S"]


def _causal_mask_templates():
    # masked (NEGM) iff 128*r + p > j for p in [0,128), j in [0,512)
    p = np.arange(128)[:, None]
    j = np.arange(512)[None, :]
    out = np.zeros((128, 4, 512), np.float32)
    for r in range(4):
        out[:, r, :] = np.where(128 * r + p > j, NEGM, 0.0).astype(np.float32)
    return np.ascontiguousarray(out.reshape(128, 4 * 512))


def _rope_cache_np():
    # mirrors reference._rope_cache in float32
    inv_freq = (1.0 / (BASE ** (np.arange(0, D, 2, dtype=np.float32) / np.float32(D)))).astype(np.float32)
    ratio = (MIN_R + (MAX_R - MIN_R) * (np.arange(H, dtype=np.float32) / np.float32(H))).astype(np.float32)
    t = (np.arange(S, dtype=np.float32)[None, :] / ratio[:, None]).astype(np.float32)
    freqs = (t[:, :, None] * inv_freq[None, None, :]).astype(np.float32)
    emb = np.concatenate([freqs, freqs], axis=-1)
    return np.cos(emb).astype(np.float32), np.sin(emb).astype(np.float32)


def _head_order(hs, Wq, Wk):
    """Exact head-outlier ordering from the last pre-RoPE attention row,
    computed in f64 on the host: srow_h = hs @ (Wk_h^T (Wq_h hs[-1]))."""
    hs64 = hs.astype(np.float64)
    q_last = hs64[-1] @ Wq.T.astype(np.float64)                 # [HID]
    Wk64 = Wk.astype(np.float64)
    Wall = np.empty((HID, H), np.float64)
    for h in range(H):
        rows = slice(h * D, (h + 1) * D)
        Wall[:, h] = Wk64[rows, :].T @ q_last[rows]
    srow = (hs64 @ Wall).T                                      # [H, S]
    sc = srow * SCALE
    m = sc.max(axis=-1, keepdims=True)
    e = np.exp(sc - m)
    aw = e / e.sum(axis=-1, keepdims=True)
    avg = aw.mean(axis=-1, keepdims=True)
    cnt = (aw > 3.0 * avg).sum(axis=-1)
    outlier = (-(cnt / np.float32(S))).astype(np.float32)
    return np.argsort(outlier, kind="stable")


def kernel(hidden_states, position_ids, Wq, Wk, Wv, Wo):
    hs = np.asarray(hidden_states, dtype=np.float32)[0]        # [S, HID]
    pos = np.asarray(position_ids).astype(np.int64)[0]         # [S]
    Wq = np.asarray(Wq, dtype=np.float32)
    Wk = np.asarray(Wk, dtype=np.float32)
    Wv = np.asarray(Wv, dtype=np.float32)
    Wo = np.asarray(Wo, dtype=np.float32)

    # ---- host: head order (exact control flow), permuted RoPE caches ----
    head_order = _head_order(hs, Wq, Wk)
    cos, sin = _rope_cache_np()
    cos_o = cos[head_order][:, pos, :]                         # [H, S, D]
    sin_o = sin[head_order][:, pos, :]
    masks = _causal_mask_templates()

    hsT = np.ascontiguousarray(hs.T).astype(NPBF16)            # [HID, S] bf16

    nc = _get_nc()
    in_maps = []
    for c in range(NCORES):
        rows = slice(c * JC, (c + 1) * JC)
        ct = np.ascontiguousarray(
            np.concatenate([cos_o[c * HPC + i].T for i in range(HPC)], axis=0)
        )  # [JC, S]
        # half-swapped signed sin: spre[0:64] = +sin[64:128],
        # spre[64:128] = -sin[0:64] (per head) — see RoPE comment in build()
        st = np.concatenate(
            [
                np.concatenate(
                    [sin_o[c * HPC + i].T[D // 2:], -sin_o[c * HPC + i].T[: D // 2]],
                    axis=0,
                )
                for i in range(HPC)
            ],
            axis=0,
        )
        in_maps.append(
            {
                "hsT": hsT,
                "wqT": np.ascontiguousarray(Wq[rows, :].T).astype(NPBF16),
                "wkT": np.ascontiguousarray(Wk[rows, :].T).astype(NPBF16),
                "wvT": np.ascontiguousarray(Wv[rows, :].T).astype(NPBF16),
                "woT": np.ascontiguousarray(Wo[:, rows].T).astype(NPBF16),
                "cosT": ct.astype(NPBF16),
                "shatT": np.ascontiguousarray(st).astype(NPBF16),
                "masks": masks,
            }
        )
    res = bass_utils.run_bass_kernel_spmd(
        nc, in_maps, core_ids=list(range(NCORES)), trace=TRACE
    )
    if TRACE:
        LAST_PROFILE["S"] = res

    # ---- host: unshard (sum o_proj partials) ----
    acc = np.zeros((HID, S), np.float64)
    for c in range(NCORES):
        acc += res.results[c]["oT"].astype(np.float64)
    return np.ascontiguousarray(acc.T)[None, :, :].astype(np.float32)
